# revision 52
# baseline (speedup 1.0000x reference)
"""Bass/Trainium2 kernel for nn_Attention_46566035423948.

Multi-head attention (B=4, N=2048, C=1024, H=16) on 8 NeuronCores.
Sharding: core c = (batch b = c//2, head-group g = c%2, 8 heads each).
Each core computes a partial projection output [N, C]; the host sums the
two head-group partials per batch and adds b_proj.

Per-core dataflow (everything in "key/channel-on-partition" layout so the
softmax denominator is a matmul reduction):
  phase 1: Q^T,K^T [512, 2048] fp32r (head pairs packed 64+64 in partition
           chunks), V [2048, 8*65] natural layout bf16 with a ones column
           per head, from bf16 xT and the W_qkv slices (SCALE pre-folded
           into Wq on host).
  phase 2: per (q-block 512, head-pair, k-chunk 128):
           S^T = K^T.T @ Q^T (row-tiled pair of fp32r matmuls)
           P^T = exp(S^T) * maskT (ScalarE exp PSUM->SBUF bf16, DVE mask)
           U  += V_aug.T @ P^T (M=65: row 64 accumulates the softmax
           denominators for free), then xn^T = U * broadcast(1/U[64]).
           Phase 1 is interleaved pair-by-pair under q-blocks 0-1 so the
           ScalarE exp pipeline (the throughput floor) starts early.
  phase 3: out = xn^T.T @ W_proj_slice (fp32r), staged through SBUF to
           DRAM. Proj groups for completed q-blocks are streamed into the
           last head-pair's k-loops; only the final q-block's groups run
           as a tail.
"""

import numpy as np
import ml_dtypes

import concourse.mybir as mybir
import concourse.tile as tile
from concourse import bacc
from concourse import bass_utils

N_CORES = 8
B, N, C, H = 4, 2048, 1024, 16
HS = C // H           # 64
SCALE = HS ** -0.5
HPC = 8               # heads per core
GW = HPC * HS         # 512: per-core head-group width
PAIRS = 4             # head pairs per core
CC = C // 128         # 8 contraction chunks over C
KC = N // 128         # 16 key chunks
QB = N // 512         # 4 query blocks of 512
QC = N // 128         # 16 query chunks of 128 (proj)

F32 = mybir.dt.float32
F32R = mybir.dt.float32r
BF16 = mybir.dt.bfloat16
EXP = mybir.ActivationFunctionType.Exp

_NC_CACHE = []


def _load_mask(nc, m_pool, mT, qb):
    """DMA the 16 [128, 512] bf16 mask tiles for one q-block."""
    tiles = []
    for kc in range(KC):
        mt = m_pool.tile([128, 512], BF16, name="m_t", tag="m_t")
        nc.sync.dma_start(mt[:], mT[qb, kc])
        tiles.append(mt)
    return tiles


def _phase2_block(nc, qb, pair, qkt, v_t, xn, mtiles,
                  s_pool, u_pool, e_pool, p_pool,
                  rinv_pool, b_pool, pre_kc=None):
    """Attention for one (q-block, head-pair): S^T, exp, mask, augmented PV
    (which also accumulates the softmax denominators in U row 64), then
    normalize into xn[pair][:, qb*512:(qb+1)*512]."""
    qs = slice(qb * 512, (qb + 1) * 512)
    KT = qkt[("k", pair)]
    QT = qkt[("q", pair)]
    U = u_pool.tile([128, 1024], F32, name="U", tag="U")
    h0 = 2 * pair * (HS + 1)
    h1 = (2 * pair + 1) * (HS + 1)
    SKEW = 5  # PV trails S by 5 k-chunks so the first S's of a unit issue
    # before PV(kc=0) blocks the PE stream on the previous unit's normalize
    p_tiles = {}

    def emit_pv(kc):
        P = p_tiles.pop(kc)
        nc.tensor.matmul(
            U[0:65, 0:512], v_t[kc][:, h0:h0 + 65],
            P[:, 0:512], start=(kc == 0), stop=(kc == KC - 1),
            tile_position=(0, 0), skip_group_check=True)
        nc.tensor.matmul(
            U[0:65, 512:1024], v_t[kc][:, h1:h1 + 65],
            P[:, 512:1024], start=(kc == 0), stop=(kc == KC - 1),
            tile_position=(0, 0), skip_group_check=True)

    for kc in range(KC + SKEW):
        if kc < KC:
            if pre_kc is not None:
                pre_kc(kc)
            ks = slice(kc * 128, (kc + 1) * 128)
            mt = mtiles[kc]
            S = s_pool.tile([128, 1024], F32, name="S", tag="S")
            nc.tensor.matmul(S[:, 0:512], KT[0:64, ks], QT[0:64, qs],
                             start=True, stop=True, tile_position=(0, 0))
            nc.tensor.matmul(S[:, 512:1024], KT[64:128, ks], QT[64:128, qs],
                             start=True, stop=True, tile_position=(64, 0))
            E = e_pool.tile([128, 1024], BF16, name="E", tag="E")
            nc.scalar.activation(E[:], S[:], EXP)
            P = p_pool.tile([128, 1024], BF16, name="P", tag="P")
            nc.vector.tensor_mul(P[:, 0:512], E[:, 0:512], mt[:])
            nc.vector.tensor_mul(P[:, 512:1024], E[:, 512:1024], mt[:])
            p_tiles[kc] = P
        if kc >= SKEW:
            emit_pv(kc - SKEW)
    r01 = rinv_pool.tile([1, 1024], F32, name="r01", tag="r01")
    nc.vector.reciprocal(r01[:], U[64:65, :])
    Bc = b_pool.tile([128, 1024], F32, name="Bc", tag="Bc")
    nc.gpsimd.partition_broadcast(Bc[:], r01[:])
    nc.vector.tensor_mul(xn[pair][0:64, qs], U[0:64, 0:512], Bc[0:64, 0:512])
    nc.vector.tensor_mul(xn[pair][64:128, qs], U[0:64, 512:1024],
                         Bc[0:64, 512:1024])


def _emit(tc, xT, wq, wk, wv, mT, wp, out):
    nc = tc.nc
    from contextlib import ExitStack

    with ExitStack() as stack:
        # persistent pools: V lives through phase 2, xn through phase 3
        v_pool = stack.enter_context(tc.tile_pool(name="vp", bufs=KC))
        xn_pool = stack.enter_context(tc.tile_pool(name="xn", bufs=1))
        wp_pool = stack.enter_context(tc.tile_pool(name="wpp", bufs=PAIRS))
        ostage_pool = stack.enter_context(tc.tile_pool(name="ostage", bufs=4))

        v_t = []
        xn = [xn_pool.tile([128, N], F32R, name=f"xn{i}", tag=f"xn{i}")
              for i in range(PAIRS)]
        wp_t = []

        # single fused region: QKV production for (pair, qb) is interleaved
        # directly ahead of the attention block that consumes it, so the
        # ScalarE exp stream starts almost immediately and PE fills ACT
        # stalls with projection work throughout.
        with tc.tile_pool(name="qkt", bufs=4) as qkt_pool, \
             tc.tile_pool(name="ep", bufs=4) as e_pool, \
             tc.tile_pool(name="pp", bufs=7) as p_pool, \
             tc.tile_pool(name="rinv", bufs=2) as rinv_pool, \
             tc.tile_pool(name="binv", bufs=2) as b_pool, \
             tc.tile_pool(name="xt", bufs=1) as xt_pool, \
             tc.tile_pool(name="wqk", bufs=20) as wqk_pool, \
             tc.tile_pool(name="wvp", bufs=1) as wv_pool, \
             tc.tile_pool(name="mp", bufs=8) as m_pool, \
             tc.tile_pool(name="ps2s", bufs=2, space="PSUM") as s_pool, \
             tc.tile_pool(name="ps1", bufs=2, space="PSUM") as ps1_pool, \
             tc.tile_pool(name="ps2u", bufs=1, space="PSUM") as u_pool:

            def dma_wqk(pair):
                wts = {}
                for which, wsrc in (("k", wk), ("q", wq)):
                    for cc in range(CC):
                        wt = wqk_pool.tile([128, 128], BF16, name="wqk_t",
                                           tag="wqk_t")
                        nc.sync.dma_start(wt[:], wsrc[pair, cc])
                        wts[(which, cc)] = wt
                return wts

            # DMA in PE-consumption order: pair-0 K weights, the 8 qb=0
            # xt chunks and wv first — the first K-group starts after
            # ~0.75 MB of DMA.
            def dma_w(which, wsrc, pair, wts):
                for cc in range(CC):
                    wt = wqk_pool.tile([128, 128], BF16, name="wqk_t",
                                       tag="wqk_t")
                    nc.sync.dma_start(wt[:], wsrc[pair, cc])
                    wts[(which, cc)] = wt

            wts0 = {}
            dma_w("k", wk, 0, wts0)
            xt_q = {}
            wv_t = []
            for cc in range(CC):
                t = xt_pool.tile([128, 512], BF16, name=f"xt{cc}_0")
                nc.sync.dma_start(t[:], xT[0, cc])
                xt_q[(cc, 0)] = t
            dma_w("q", wq, 0, wts0)
            for cc in range(CC):
                t = wv_pool.tile([128, GW], BF16, name=f"wv{cc}")
                nc.sync.dma_start(t[:], wv[cc * 128:(cc + 1) * 128, :])
                wv_t.append(t)
            for qb in range(1, QB):
                for cc in range(CC):
                    t = xt_pool.tile([128, 512], BF16, name=f"xt{cc}_{qb}")
                    nc.sync.dma_start(t[:], xT[qb, cc])
                    xt_q[(cc, qb)] = t

            # V tiles are emitted lazily inside the first attention block's
            # k-loop so ScalarE's exp pipeline starts early. Layout
            # [128, 8*65]: head h at cols h*65..h*65+64 plus a ones column
            # at h*65+64, so PV matmuls (M=65) also produce the softmax
            # row sums.
            def emit_v(kc):
                ps = ps1_pool.tile([128, 512], F32, name="ps1t", tag="ps1t")
                for cc in range(CC):
                    nc.tensor.matmul(
                        ps[:],
                        xt_q[(cc, kc // 4)][:, (kc % 4) * 128:
                                            (kc % 4) * 128 + 128],
                        wv_t[cc][:],
                        start=(cc == 0), stop=(cc == CC - 1))
                t = v_pool.tile([128, HPC * (HS + 1)], BF16, name="v_t",
                                tag="v_t")
                tv = t[:].rearrange("p (h d) -> p h d", h=HPC)
                nc.gpsimd.memset(tv[:, :, HS:HS + 1], 1.0)
                nc.vector.tensor_copy(
                    tv[:, :, 0:HS],
                    ps[:].rearrange("p (h d) -> p h d", h=HPC))
                v_t.append(t)

            proj_done = []

            def proj_group(qc, nh, pool=None):
                pool = pool if pool is not None else ps1_pool
                ps = pool.tile([128, 512], F32, name="ps1t", tag="ps1t")
                for pair_ in range(PAIRS):
                    nc.tensor.matmul(
                        ps[:], xn[pair_][:, qc * 128:(qc + 1) * 128],
                        wp_t[pair_][:, nh * 512:(nh + 1) * 512],
                        start=(pair_ == 0), stop=(pair_ == PAIRS - 1))
                ost = ostage_pool.tile([128, 512], F32, name="ost", tag="ost")
                nc.scalar.copy(ost[:], ps[:])
                nc.sync.dma_start(
                    out[qc * 128:(qc + 1) * 128, nh * 512:(nh + 1) * 512],
                    ost[:])
                proj_done.append((qc, nh))

            for pair in range(PAIRS):
                wts = wts0 if pair == 0 else dma_wqk(pair)
                qkt = {}
                for which in ("q", "k"):
                    qkt[(which, pair)] = qkt_pool.tile(
                        [128, N], F32R, name="qkt_t", tag="qkt_t")
                if pair == 1:
                    # prefetch proj weights once SBUF headroom exists
                    for pp_ in range(PAIRS):
                        t = wp_pool.tile([128, C], F32R, name="wp_t",
                                         tag="wp_t")
                        nc.sync.dma_start(
                            t[:], wp[pp_ * 128:(pp_ + 1) * 128, :])
                        wp_t.append(t)
                def qk_group(which, qb):
                    dst = qkt[(which, pair)]
                    ps = ps1_pool.tile([128, 512], F32, name="ps1t",
                                       tag="ps1t")
                    for cc in range(CC):
                        nc.tensor.matmul(
                            ps[:], wts[(which, cc)][:],
                            xt_q[(cc, qb)][:],
                            start=(cc == 0), stop=(cc == CC - 1))
                    nc.vector.tensor_copy(
                        dst[:, qb * 512:(qb + 1) * 512], ps[:])

                # K^T is contracted over ALL key blocks by every attention
                # block, so it must be complete before (or produced just
                # ahead of) the k-chunks that read it. For pair 0 the later
                # K-groups are injected into the first block's k-loop (with
                # V) so the exp stream starts after ~2 QK groups, not 5.
                if pair == 0:
                    qk_group("k", 0)

                    def pre0(kc):
                        if kc in (1, 5, 9):
                            qk_group("k", kc // 4 + 1)
                        emit_v(kc)
                else:
                    for qb in range(QB):
                        qk_group("k", qb)
                    pre0 = None
                for qb in range(QB):
                    qk_group("q", qb)
                    if pair == 0 and qb == 0:
                        pre = pre0
                    elif pair == PAIRS - 1 and qb >= 1:
                        # last pair: the q-blocks processed so far have
                        # complete xn across all pairs — stream their proj
                        # groups into this unit's k-loop (PE slack fills
                        # while ACT stays the critical engine)
                        pend = [(c, n) for c in range((qb - 1) * 4, qb * 4)
                                for n in range(2)][:5]

                        def pre(kc, _p=pend):
                            if _p and kc % 3 == 1:
                                proj_group(*_p.pop(0))
                    else:
                        pre = None
                    mtiles = _load_mask(nc, m_pool, mT, qb)
                    _phase2_block(nc, qb, pair, qkt, v_t, xn, mtiles,
                                  s_pool, u_pool,
                                  e_pool, p_pool, rinv_pool, b_pool,
                                  pre_kc=pre)

            # remaining proj groups (qb3's q-chunks + any not streamed)
            done = set(proj_done)
            for qc in range(QC):
                for nh in range(2):
                    if (qc, nh) not in done:
                        proj_group(qc, nh)


def build():
    if _NC_CACHE:
        return _NC_CACHE[0]
    nc = bacc.Bacc("TRN2", target_bir_lowering=False, debug=False,
                   enable_asserts=False, num_devices=N_CORES)
    xT = nc.dram_tensor("xT", [QB, CC, 128, 512], BF16,
                        kind="ExternalInput").ap()
    wq = nc.dram_tensor("wq", [PAIRS, CC, 128, 128], BF16,
                        kind="ExternalInput").ap()
    wk = nc.dram_tensor("wk", [PAIRS, CC, 128, 128], BF16,
                        kind="ExternalInput").ap()
    wv = nc.dram_tensor("wv", [C, GW], BF16, kind="ExternalInput").ap()
    mT = nc.dram_tensor("mT", [QB, KC, 128, 512], BF16,
                        kind="ExternalInput").ap()
    wp = nc.dram_tensor("wp", [GW, C], F32R, kind="ExternalInput").ap()
    out = nc.dram_tensor("out", [N, C], F32, kind="ExternalOutput").ap()
    with tile.TileContext(nc) as tc:
        _emit(tc, xT, wq, wk, wv, mT, wp, out)
    nc.compile()
    _NC_CACHE.append(nc)
    return nc


def _tile4(a, rows, cols):
    """[R, Q] -> [Q//cols, R//rows, rows, cols] contiguous tiles so every
    device DMA is a single contiguous transfer."""
    R, Q = a.shape
    return np.ascontiguousarray(
        a.reshape(R // rows, rows, Q // cols, cols).transpose(0, 2, 1, 3)
         .transpose(1, 0, 2, 3))


def shard_inputs(joint_feature, mask, W_qkv, W_proj, b_proj):
    mT = _tile4(np.ascontiguousarray(mask[0, 0].T).astype(ml_dtypes.bfloat16),
                128, 512)
    in_maps = []
    for c in range(N_CORES):
        b, g = divmod(c, 2)
        lo, hi = g * GW, (g + 1) * GW
        in_maps.append({
            "xT": _tile4(np.ascontiguousarray(joint_feature[b].T)
                         .astype(ml_dtypes.bfloat16), 128, 512),
            "wq": _tile4((W_qkv[:, lo:hi] * SCALE)
                         .astype(ml_dtypes.bfloat16), 128, 128),
            "wk": _tile4(W_qkv[:, C + lo:C + hi]
                         .astype(ml_dtypes.bfloat16), 128, 128),
            "wv": np.ascontiguousarray(W_qkv[:, 2 * C + lo:2 * C + hi]).astype(ml_dtypes.bfloat16),
            "mT": mT,
            "wp": np.ascontiguousarray(W_proj[lo:hi, :]).astype(np.float32),
        })
    return in_maps


def kernel(joint_feature, mask, W_qkv, W_proj, b_proj):
    joint_feature = np.asarray(joint_feature, dtype=np.float32)
    mask = np.asarray(mask)
    W_qkv = np.asarray(W_qkv, dtype=np.float32)
    W_proj = np.asarray(W_proj, dtype=np.float32)
    b_proj = np.asarray(b_proj, dtype=np.float32)

    nc = build()
    in_maps = shard_inputs(joint_feature, mask, W_qkv, W_proj, b_proj)
    res = bass_utils.run_bass_kernel_spmd(nc, in_maps,
                                          core_ids=list(range(N_CORES)))
    out = np.empty((B, N, C), dtype=np.float32)
    for b in range(B):
        out[b] = res.results[2 * b]["out"] + res.results[2 * b + 1]["out"] \
            + b_proj
    return out


# revision 62
# speedup vs baseline: 1.0063x; 1.0063x over previous
"""Bass/Trainium2 kernel for nn_Attention_46566035423948.

Multi-head attention (B=4, N=2048, C=1024, H=16) on 8 NeuronCores.
Sharding: core c = (batch b = c//2, head-group g = c%2, 8 heads each).
Each core computes a partial projection output [N, C]; the host sums the
two head-group partials per batch and adds b_proj.

Per-core dataflow (everything in "key/channel-on-partition" layout so the
softmax denominator is a matmul reduction):
  phase 1: Q^T,K^T [512, 2048] fp32r (head pairs packed 64+64 in partition
           chunks), V [2048, 8*65] natural layout bf16 with a ones column
           per head, from bf16 xT and the W_qkv slices (SCALE pre-folded
           into Wq on host).
  phase 2: per (q-block 512, head-pair, k-chunk 128):
           S^T = K^T.T @ Q^T (row-tiled pair of fp32r matmuls)
           P^T = exp(S^T) * maskT (ScalarE exp PSUM->SBUF bf16, DVE mask)
           U  += V_aug.T @ P^T (M=65: row 64 accumulates the softmax
           denominators for free), then xn^T = U * broadcast(1/U[64]).
           Phase 1 is interleaved pair-by-pair under q-blocks 0-1 so the
           ScalarE exp pipeline (the throughput floor) starts early.
  phase 3: out = xn^T.T @ W_proj_slice (fp32r), staged through SBUF to
           DRAM. Proj groups for completed q-blocks are streamed into the
           last head-pair's k-loops; only the final q-block's groups run
           as a tail.
"""

import numpy as np
import ml_dtypes

import concourse.mybir as mybir
import concourse.tile as tile
from concourse import bacc
from concourse import bass_utils

N_CORES = 8
B, N, C, H = 4, 2048, 1024, 16
HS = C // H           # 64
SCALE = HS ** -0.5
HPC = 8               # heads per core
GW = HPC * HS         # 512: per-core head-group width
PAIRS = 4             # head pairs per core
CC = C // 128         # 8 contraction chunks over C
KC = N // 128         # 16 key chunks
QB = N // 512         # 4 query blocks of 512
QC = N // 128         # 16 query chunks of 128 (proj)

F32 = mybir.dt.float32
F32R = mybir.dt.float32r
BF16 = mybir.dt.bfloat16
EXP = mybir.ActivationFunctionType.Exp

_NC_CACHE = []


def _load_mask(nc, m_pool, mT, qb):
    """DMA the 16 [128, 512] bf16 mask tiles for one q-block."""
    tiles = []
    for kc in range(KC):
        mt = m_pool.tile([128, 512], BF16, name="m_t", tag="m_t")
        nc.sync.dma_start(mt[:], mT[qb, kc])
        tiles.append(mt)
    return tiles


def _phase2_block(nc, qb, pair, qkt, v_t, xn, mtiles,
                  s_pool, u_pool, e_pool, p_pool,
                  rinv_pool, b_pool, pre_kc=None):
    """Attention for one (q-block, head-pair): S^T, exp, mask, augmented PV
    (which also accumulates the softmax denominators in U row 64), then
    normalize into xn[pair][:, qb*512:(qb+1)*512]."""
    qs = slice(qb * 512, (qb + 1) * 512)
    KT = qkt[("k", pair)]
    QT = qkt[("q", pair)]
    U = u_pool.tile([128, 1024], F32, name="U", tag="U")
    h0 = 2 * pair * (HS + 1)
    h1 = (2 * pair + 1) * (HS + 1)
    SKEW = 5  # PV trails S by 5 k-chunks so the first S's of a unit issue
    # before PV(kc=0) blocks the PE stream on the previous unit's normalize
    p_tiles = {}

    def emit_pv(kc):
        P = p_tiles.pop(kc)
        nc.tensor.matmul(
            U[0:65, 0:512], v_t[kc][:, h0:h0 + 65],
            P[:, 0:512], start=(kc == 0), stop=(kc == KC - 1),
            tile_position=(0, 0), skip_group_check=True)
        nc.tensor.matmul(
            U[0:65, 512:1024], v_t[kc][:, h1:h1 + 65],
            P[:, 512:1024], start=(kc == 0), stop=(kc == KC - 1),
            tile_position=(0, 0), skip_group_check=True)

    for kc in range(KC + SKEW):
        if kc < KC:
            if pre_kc is not None:
                pre_kc(kc)
            ks = slice(kc * 128, (kc + 1) * 128)
            mt = mtiles[kc]
            S = s_pool.tile([128, 1024], F32, name="S", tag="S")
            nc.tensor.matmul(S[:, 0:512], KT[0:64, ks], QT[0:64, qs],
                             start=True, stop=True, tile_position=(0, 0))
            nc.tensor.matmul(S[:, 512:1024], KT[64:128, ks], QT[64:128, qs],
                             start=True, stop=True, tile_position=(64, 0))
            E = e_pool.tile([128, 1024], BF16, name="E", tag="E")
            nc.scalar.activation(E[:], S[:], EXP)
            P = p_pool.tile([128, 1024], BF16, name="P", tag="P")
            nc.vector.tensor_mul(P[:, 0:512], E[:, 0:512], mt[:])
            nc.vector.tensor_mul(P[:, 512:1024], E[:, 512:1024], mt[:])
            p_tiles[kc] = P
        if kc >= SKEW:
            emit_pv(kc - SKEW)
    r01 = rinv_pool.tile([1, 1024], F32, name="r01", tag="r01")
    nc.vector.reciprocal(r01[:], U[64:65, :])
    Bc = b_pool.tile([128, 1024], F32, name="Bc", tag="Bc")
    nc.gpsimd.partition_broadcast(Bc[:], r01[:])
    nc.vector.tensor_mul(xn[pair][0:64, qs], U[0:64, 0:512], Bc[0:64, 0:512])
    nc.vector.tensor_mul(xn[pair][64:128, qs], U[0:64, 512:1024],
                         Bc[0:64, 512:1024])


def _emit(tc, xT, wq, wk, wv, mT, wp, out):
    nc = tc.nc
    from contextlib import ExitStack

    with ExitStack() as stack:
        # persistent pools: V lives through phase 2, xn through phase 3
        v_pool = stack.enter_context(tc.tile_pool(name="vp", bufs=KC))
        xn_pool = stack.enter_context(tc.tile_pool(name="xn", bufs=1))
        wp_pool = stack.enter_context(tc.tile_pool(name="wpp", bufs=PAIRS))
        ostage_pool = stack.enter_context(tc.tile_pool(name="ostage", bufs=6))

        v_t = []
        xn = [xn_pool.tile([128, N], F32R, name=f"xn{i}", tag=f"xn{i}")
              for i in range(PAIRS)]
        wp_t = []

        # single fused region: QKV production for (pair, qb) is interleaved
        # directly ahead of the attention block that consumes it, so the
        # ScalarE exp stream starts almost immediately and PE fills ACT
        # stalls with projection work throughout.
        with tc.tile_pool(name="qkt", bufs=4) as qkt_pool, \
             tc.tile_pool(name="ep", bufs=4) as e_pool, \
             tc.tile_pool(name="pp", bufs=7) as p_pool, \
             tc.tile_pool(name="rinv", bufs=2) as rinv_pool, \
             tc.tile_pool(name="binv", bufs=2) as b_pool, \
             tc.tile_pool(name="xt", bufs=1) as xt_pool, \
             tc.tile_pool(name="wqk", bufs=20) as wqk_pool, \
             tc.tile_pool(name="wvp", bufs=1) as wv_pool, \
             tc.tile_pool(name="mp", bufs=8) as m_pool, \
             tc.tile_pool(name="ps2s", bufs=2, space="PSUM") as s_pool, \
             tc.tile_pool(name="ps1", bufs=2, space="PSUM") as ps1_pool, \
             tc.tile_pool(name="ps2u", bufs=1, space="PSUM") as u_pool:

            def dma_wqk(pair):
                wts = {}
                for which, wsrc in (("k", wk), ("q", wq)):
                    for cc in range(CC):
                        wt = wqk_pool.tile([128, 128], BF16, name="wqk_t",
                                           tag="wqk_t")
                        nc.sync.dma_start(wt[:], wsrc[pair, cc])
                        wts[(which, cc)] = wt
                return wts

            # DMA in PE-consumption order: pair-0 K weights, the 8 qb=0
            # xt chunks and wv first — the first K-group starts after
            # ~0.75 MB of DMA.
            def dma_w(which, wsrc, pair, wts):
                for cc in range(CC):
                    wt = wqk_pool.tile([128, 128], BF16, name="wqk_t",
                                       tag="wqk_t")
                    nc.sync.dma_start(wt[:], wsrc[pair, cc])
                    wts[(which, cc)] = wt

            wts0 = {}
            dma_w("k", wk, 0, wts0)
            xt_q = {}
            wv_t = []
            for cc in range(CC):
                t = xt_pool.tile([128, 512], BF16, name=f"xt{cc}_0")
                nc.sync.dma_start(t[:], xT[0, cc])
                xt_q[(cc, 0)] = t
            dma_w("q", wq, 0, wts0)
            for cc in range(CC):
                t = wv_pool.tile([128, GW], BF16, name=f"wv{cc}")
                nc.sync.dma_start(t[:], wv[cc * 128:(cc + 1) * 128, :])
                wv_t.append(t)
            for qb in range(1, QB):
                for cc in range(CC):
                    t = xt_pool.tile([128, 512], BF16, name=f"xt{cc}_{qb}")
                    nc.sync.dma_start(t[:], xT[qb, cc])
                    xt_q[(cc, qb)] = t

            # V tiles are emitted lazily inside the first attention block's
            # k-loop so ScalarE's exp pipeline starts early. Layout
            # [128, 8*65]: head h at cols h*65..h*65+64 plus a ones column
            # at h*65+64, so PV matmuls (M=65) also produce the softmax
            # row sums.
            def emit_v(kc):
                ps = ps1_pool.tile([128, 512], F32, name="ps1t", tag="ps1t")
                for cc in range(CC):
                    nc.tensor.matmul(
                        ps[:],
                        xt_q[(cc, kc // 4)][:, (kc % 4) * 128:
                                            (kc % 4) * 128 + 128],
                        wv_t[cc][:],
                        start=(cc == 0), stop=(cc == CC - 1))
                t = v_pool.tile([128, HPC * (HS + 1)], BF16, name="v_t",
                                tag="v_t")
                tv = t[:].rearrange("p (h d) -> p h d", h=HPC)
                nc.gpsimd.memset(tv[:, :, HS:HS + 1], 1.0)
                nc.vector.tensor_copy(
                    tv[:, :, 0:HS],
                    ps[:].rearrange("p (h d) -> p h d", h=HPC))
                v_t.append(t)

            proj_done = []

            def proj_group(qc, nh, pool=None):
                pool = pool if pool is not None else ps1_pool
                ps = pool.tile([128, 512], F32, name="ps1t", tag="ps1t")
                for pair_ in range(PAIRS):
                    nc.tensor.matmul(
                        ps[:], xn[pair_][:, qc * 128:(qc + 1) * 128],
                        wp_t[pair_][:, nh * 512:(nh + 1) * 512],
                        start=(pair_ == 0), stop=(pair_ == PAIRS - 1))
                ost = ostage_pool.tile([128, 512], F32, name="ost", tag="ost")
                nc.scalar.copy(ost[:], ps[:])
                nc.sync.dma_start(
                    out[qc * 128:(qc + 1) * 128, nh * 512:(nh + 1) * 512],
                    ost[:])
                proj_done.append((qc, nh))

            for pair in range(PAIRS):
                wts = wts0 if pair == 0 else dma_wqk(pair)
                qkt = {}
                for which in ("q", "k"):
                    qkt[(which, pair)] = qkt_pool.tile(
                        [128, N], F32R, name="qkt_t", tag="qkt_t")
                if pair == 1:
                    # prefetch proj weights once SBUF headroom exists
                    for pp_ in range(PAIRS):
                        t = wp_pool.tile([128, C], F32R, name="wp_t",
                                         tag="wp_t")
                        nc.sync.dma_start(
                            t[:], wp[pp_ * 128:(pp_ + 1) * 128, :])
                        wp_t.append(t)
                def qk_group(which, qb):
                    dst = qkt[(which, pair)]
                    ps = ps1_pool.tile([128, 512], F32, name="ps1t",
                                       tag="ps1t")
                    for cc in range(CC):
                        nc.tensor.matmul(
                            ps[:], wts[(which, cc)][:],
                            xt_q[(cc, qb)][:],
                            start=(cc == 0), stop=(cc == CC - 1))
                    nc.vector.tensor_copy(
                        dst[:, qb * 512:(qb + 1) * 512], ps[:])

                # K^T is contracted over ALL key blocks by every attention
                # block, so it must be complete before (or produced just
                # ahead of) the k-chunks that read it. For pair 0 the later
                # K-groups are injected into the first block's k-loop (with
                # V) so the exp stream starts after ~2 QK groups, not 5.
                if pair == 0:
                    qk_group("k", 0)

                    def pre0(kc):
                        if kc in (1, 5, 9):
                            qk_group("k", kc // 4 + 1)
                        emit_v(kc)
                else:
                    for qb in range(QB):
                        qk_group("k", qb)
                    pre0 = None
                for qb in range(QB):
                    qk_group("q", qb)
                    if pair == 0 and qb == 0:
                        pre = pre0
                    elif pair == PAIRS - 1 and qb >= 1:
                        # last pair: the q-blocks processed so far have
                        # complete xn across all pairs — stream their proj
                        # groups into this unit's k-loop (PE slack fills
                        # while ACT stays the critical engine)
                        done = set(proj_done)
                        lim, step = (8, 2) if qb == QB - 1 else (5, 3)
                        pend = [(c, n) for c in range(qb * 4)
                                for n in range(2) if (c, n) not in done][:lim]

                        def pre(kc, _p=pend, _s=step):
                            if _p and kc % _s == 1:
                                proj_group(*_p.pop(0))
                    else:
                        pre = None
                    mtiles = _load_mask(nc, m_pool, mT, qb)
                    _phase2_block(nc, qb, pair, qkt, v_t, xn, mtiles,
                                  s_pool, u_pool,
                                  e_pool, p_pool, rinv_pool, b_pool,
                                  pre_kc=pre)

            # remaining proj groups (qb3's q-chunks + any not streamed)
            done = set(proj_done)
            for qc in range(QC):
                for nh in range(2):
                    if (qc, nh) not in done:
                        proj_group(qc, nh)


def build():
    if _NC_CACHE:
        return _NC_CACHE[0]
    nc = bacc.Bacc("TRN2", target_bir_lowering=False, debug=False,
                   enable_asserts=False, num_devices=N_CORES)
    xT = nc.dram_tensor("xT", [QB, CC, 128, 512], BF16,
                        kind="ExternalInput").ap()
    wq = nc.dram_tensor("wq", [PAIRS, CC, 128, 128], BF16,
                        kind="ExternalInput").ap()
    wk = nc.dram_tensor("wk", [PAIRS, CC, 128, 128], BF16,
                        kind="ExternalInput").ap()
    wv = nc.dram_tensor("wv", [C, GW], BF16, kind="ExternalInput").ap()
    mT = nc.dram_tensor("mT", [QB, KC, 128, 512], BF16,
                        kind="ExternalInput").ap()
    wp = nc.dram_tensor("wp", [GW, C], F32R, kind="ExternalInput").ap()
    out = nc.dram_tensor("out", [N, C], F32, kind="ExternalOutput").ap()
    with tile.TileContext(nc) as tc:
        _emit(tc, xT, wq, wk, wv, mT, wp, out)
    nc.compile()
    _NC_CACHE.append(nc)
    return nc


def _tile4(a, rows, cols):
    """[R, Q] -> [Q//cols, R//rows, rows, cols] contiguous tiles so every
    device DMA is a single contiguous transfer."""
    R, Q = a.shape
    return np.ascontiguousarray(
        a.reshape(R // rows, rows, Q // cols, cols).transpose(0, 2, 1, 3)
         .transpose(1, 0, 2, 3))


def shard_inputs(joint_feature, mask, W_qkv, W_proj, b_proj):
    mT = _tile4(np.ascontiguousarray(mask[0, 0].T).astype(ml_dtypes.bfloat16),
                128, 512)
    in_maps = []
    for c in range(N_CORES):
        b, g = divmod(c, 2)
        lo, hi = g * GW, (g + 1) * GW
        in_maps.append({
            "xT": _tile4(np.ascontiguousarray(joint_feature[b].T)
                         .astype(ml_dtypes.bfloat16), 128, 512),
            "wq": _tile4((W_qkv[:, lo:hi] * SCALE)
                         .astype(ml_dtypes.bfloat16), 128, 128),
            "wk": _tile4(W_qkv[:, C + lo:C + hi]
                         .astype(ml_dtypes.bfloat16), 128, 128),
            "wv": np.ascontiguousarray(W_qkv[:, 2 * C + lo:2 * C + hi]).astype(ml_dtypes.bfloat16),
            "mT": mT,
            "wp": np.ascontiguousarray(W_proj[lo:hi, :]).astype(np.float32),
        })
    return in_maps


def kernel(joint_feature, mask, W_qkv, W_proj, b_proj):
    joint_feature = np.asarray(joint_feature, dtype=np.float32)
    mask = np.asarray(mask)
    W_qkv = np.asarray(W_qkv, dtype=np.float32)
    W_proj = np.asarray(W_proj, dtype=np.float32)
    b_proj = np.asarray(b_proj, dtype=np.float32)

    nc = build()
    in_maps = shard_inputs(joint_feature, mask, W_qkv, W_proj, b_proj)
    res = bass_utils.run_bass_kernel_spmd(nc, in_maps,
                                          core_ids=list(range(N_CORES)))
    out = np.empty((B, N, C), dtype=np.float32)
    for b in range(B):
        out[b] = res.results[2 * b]["out"] + res.results[2 * b + 1]["out"] \
            + b_proj
    return out


# revision 63
# speedup vs baseline: 1.0411x; 1.0346x over previous
"""Bass/Trainium2 kernel for nn_Attention_46566035423948.

Multi-head attention (B=4, N=2048, C=1024, H=16) on 8 NeuronCores.
Sharding: core c = (batch b = c//2, head-group g = c%2, 8 heads each).
Each core computes a partial projection output [N, C]; the host sums the
two head-group partials per batch and adds b_proj.

Per-core dataflow (everything in "key/channel-on-partition" layout so the
softmax denominator is a matmul reduction):
  phase 1: Q^T,K^T [512, 2048] fp32r (head pairs packed 64+64 in partition
           chunks), V [2048, 8*65] natural layout bf16 with a ones column
           per head, from bf16 xT and the W_qkv slices (SCALE pre-folded
           into Wq on host).
  phase 2: per (q-block 512, head-pair, k-chunk 128):
           S^T = K^T.T @ Q^T (row-tiled pair of fp32r matmuls)
           P^T = exp(S^T) * maskT (ScalarE exp PSUM->SBUF bf16, DVE mask)
           U  += V_aug.T @ P^T (M=65: row 64 accumulates the softmax
           denominators for free), then xn^T = U * broadcast(1/U[64]).
           Phase 1 is interleaved pair-by-pair under q-blocks 0-1 so the
           ScalarE exp pipeline (the throughput floor) starts early.
  phase 3: out = xn^T.T @ W_proj_slice (fp32r), staged through SBUF to
           DRAM. Proj groups for completed q-blocks are streamed into the
           last head-pair's k-loops; only the final q-block's groups run
           as a tail.
"""

import numpy as np
import ml_dtypes

import concourse.mybir as mybir
import concourse.tile as tile
from concourse import bacc
from concourse import bass_utils

N_CORES = 8
B, N, C, H = 4, 2048, 1024, 16
HS = C // H           # 64
SCALE = HS ** -0.5
HPC = 8               # heads per core
GW = HPC * HS         # 512: per-core head-group width
PAIRS = 4             # head pairs per core
CC = C // 128         # 8 contraction chunks over C
KC = N // 128         # 16 key chunks
QB = N // 512         # 4 query blocks of 512
QC = N // 128         # 16 query chunks of 128 (proj)

F32 = mybir.dt.float32
F32R = mybir.dt.float32r
BF16 = mybir.dt.bfloat16
EXP = mybir.ActivationFunctionType.Exp

_NC_CACHE = []


def _load_mask(nc, m_pool, mT, qb):
    """DMA the 16 [128, 512] bf16 mask tiles for one q-block."""
    tiles = []
    for kc in range(KC):
        mt = m_pool.tile([128, 512], BF16, name="m_t", tag="m_t")
        nc.sync.dma_start(mt[:], mT[qb, kc])
        tiles.append(mt)
    return tiles


def _phase2_block(nc, qb, pair, qkt, v_t, xn, mtiles,
                  s_pool, u_pool, e_pool, p_pool,
                  rinv_pool, b_pool, pre_kc=None):
    """Attention for one (q-block, head-pair): S^T, exp, mask, augmented PV
    (which also accumulates the softmax denominators in U row 64), then
    normalize into xn[pair][:, qb*512:(qb+1)*512]."""
    qs = slice(qb * 512, (qb + 1) * 512)
    KT = qkt[("k", pair)]
    QT = qkt[("q", pair)]
    U = u_pool.tile([128, 1024], F32, name="U", tag="U")
    h0 = 2 * pair * (HS + 1)
    h1 = (2 * pair + 1) * (HS + 1)
    SKEW = 5  # PV trails S by 5 k-chunks so the first S's of a unit issue
    # before PV(kc=0) blocks the PE stream on the previous unit's normalize
    p_tiles = {}

    def emit_pv(kc):
        P = p_tiles.pop(kc)
        nc.tensor.matmul(
            U[0:65, 0:512], v_t[kc][:, h0:h0 + 65],
            P[:, 0:512], start=(kc == 0), stop=(kc == KC - 1),
            tile_position=(0, 0), skip_group_check=True)
        nc.tensor.matmul(
            U[0:65, 512:1024], v_t[kc][:, h1:h1 + 65],
            P[:, 512:1024], start=(kc == 0), stop=(kc == KC - 1),
            tile_position=(0, 0), skip_group_check=True)

    for kc in range(KC + SKEW):
        if kc < KC:
            if pre_kc is not None:
                pre_kc(kc)
            ks = slice(kc * 128, (kc + 1) * 128)
            mt = mtiles[kc]
            S = s_pool.tile([128, 1024], F32, name="S", tag="S")
            nc.tensor.matmul(S[:, 0:512], KT[0:64, ks], QT[0:64, qs],
                             start=True, stop=True, tile_position=(0, 0))
            nc.tensor.matmul(S[:, 512:1024], KT[64:128, ks], QT[64:128, qs],
                             start=True, stop=True, tile_position=(64, 0))
            E = e_pool.tile([128, 1024], BF16, name="E", tag="E")
            nc.scalar.activation(E[:], S[:], EXP)
            P = p_pool.tile([128, 1024], BF16, name="P", tag="P")
            nc.vector.tensor_mul(P[:, 0:512], E[:, 0:512], mt[:])
            nc.vector.tensor_mul(P[:, 512:1024], E[:, 512:1024], mt[:])
            p_tiles[kc] = P
        if kc >= SKEW:
            emit_pv(kc - SKEW)
    r01 = rinv_pool.tile([1, 1024], F32, name="r01", tag="r01")
    nc.vector.reciprocal(r01[:], U[64:65, :])
    Bc = b_pool.tile([128, 1024], F32, name="Bc", tag="Bc")
    nc.gpsimd.partition_broadcast(Bc[:], r01[:])
    nc.vector.tensor_mul(xn[pair][0:64, qs], U[0:64, 0:512], Bc[0:64, 0:512])
    nc.vector.tensor_mul(xn[pair][64:128, qs], U[0:64, 512:1024],
                         Bc[0:64, 512:1024])


def _emit(tc, xT, wq, wk, wv, mT, wp, out):
    nc = tc.nc
    from contextlib import ExitStack

    with ExitStack() as stack:
        # persistent pools: V lives through phase 2, xn through phase 3
        v_pool = stack.enter_context(tc.tile_pool(name="vp", bufs=KC))
        xn_pool = stack.enter_context(tc.tile_pool(name="xn", bufs=1))
        wp_pool = stack.enter_context(tc.tile_pool(name="wpp", bufs=PAIRS))
        ostage_pool = stack.enter_context(tc.tile_pool(name="ostage", bufs=6))

        v_t = []
        xn = [xn_pool.tile([128, N], F32R, name=f"xn{i}", tag=f"xn{i}")
              for i in range(PAIRS)]
        wp_t = []

        # single fused region: QKV production for (pair, qb) is interleaved
        # directly ahead of the attention block that consumes it, so the
        # ScalarE exp stream starts almost immediately and PE fills ACT
        # stalls with projection work throughout.
        with tc.tile_pool(name="qkt", bufs=4) as qkt_pool, \
             tc.tile_pool(name="ep", bufs=4) as e_pool, \
             tc.tile_pool(name="pp", bufs=7) as p_pool, \
             tc.tile_pool(name="rinv", bufs=2) as rinv_pool, \
             tc.tile_pool(name="binv", bufs=2) as b_pool, \
             tc.tile_pool(name="xt", bufs=1) as xt_pool, \
             tc.tile_pool(name="wqk", bufs=4) as wqk_pool, \
             tc.tile_pool(name="wvp", bufs=1) as wv_pool, \
             tc.tile_pool(name="mp", bufs=8) as m_pool, \
             tc.tile_pool(name="ps2s", bufs=2, space="PSUM") as s_pool, \
             tc.tile_pool(name="ps1", bufs=2, space="PSUM") as ps1_pool, \
             tc.tile_pool(name="ps2u", bufs=1, space="PSUM") as u_pool:

            def dma_wqk(pair):
                wts = {}
                for which, wsrc in (("k", wk), ("q", wq)):
                    wt = wqk_pool.tile([128, CC * 128], BF16, name="wqk_t",
                                       tag="wqk_t")
                    nc.sync.dma_start(wt[:], wsrc[pair])
                    for cc in range(CC):
                        wts[(which, cc)] = wt[:, cc * 128:(cc + 1) * 128]
                return wts

            # DMA in PE-consumption order: pair-0 K weights, the 8 qb=0
            # xt chunks and wv first — the first K-group starts after
            # ~0.75 MB of DMA.
            def dma_w(which, wsrc, pair, wts):
                wt = wqk_pool.tile([128, CC * 128], BF16, name="wqk_t",
                                   tag="wqk_t")
                nc.sync.dma_start(wt[:], wsrc[pair])
                for cc in range(CC):
                    wts[(which, cc)] = wt[:, cc * 128:(cc + 1) * 128]

            wts0 = {}
            dma_w("k", wk, 0, wts0)
            xt_q = {}
            wv_t = []
            t = xt_pool.tile([128, CC * 512], BF16, name="xt_0")
            nc.sync.dma_start(t[:], xT[0])
            for cc in range(CC):
                xt_q[(cc, 0)] = t[:, cc * 512:(cc + 1) * 512]
            dma_w("q", wq, 0, wts0)
            t = wv_pool.tile([128, CC * 512], BF16, name="wv_all")
            nc.sync.dma_start(t[:], wv[:])
            for cc in range(CC):
                wv_t.append(t[:, cc * 512:(cc + 1) * 512])
            for qb in range(1, QB):
                t = xt_pool.tile([128, CC * 512], BF16, name=f"xt_{qb}")
                nc.sync.dma_start(t[:], xT[qb])
                for cc in range(CC):
                    xt_q[(cc, qb)] = t[:, cc * 512:(cc + 1) * 512]

            # V tiles are emitted lazily inside the first attention block's
            # k-loop so ScalarE's exp pipeline starts early. Layout
            # [128, 8*65]: head h at cols h*65..h*65+64 plus a ones column
            # at h*65+64, so PV matmuls (M=65) also produce the softmax
            # row sums.
            def emit_v(kc):
                ps = ps1_pool.tile([128, 512], F32, name="ps1t", tag="ps1t")
                for cc in range(CC):
                    nc.tensor.matmul(
                        ps[:],
                        xt_q[(cc, kc // 4)][:, (kc % 4) * 128:
                                            (kc % 4) * 128 + 128],
                        wv_t[cc],
                        start=(cc == 0), stop=(cc == CC - 1))
                t = v_pool.tile([128, HPC * (HS + 1)], BF16, name="v_t",
                                tag="v_t")
                tv = t[:].rearrange("p (h d) -> p h d", h=HPC)
                nc.gpsimd.memset(tv[:, :, HS:HS + 1], 1.0)
                nc.vector.tensor_copy(
                    tv[:, :, 0:HS],
                    ps[:].rearrange("p (h d) -> p h d", h=HPC))
                v_t.append(t)

            proj_done = []

            def proj_group(qc, nh, pool=None):
                pool = pool if pool is not None else ps1_pool
                ps = pool.tile([128, 512], F32, name="ps1t", tag="ps1t")
                for pair_ in range(PAIRS):
                    nc.tensor.matmul(
                        ps[:], xn[pair_][:, qc * 128:(qc + 1) * 128],
                        wp_t[pair_][:, nh * 512:(nh + 1) * 512],
                        start=(pair_ == 0), stop=(pair_ == PAIRS - 1))
                ost = ostage_pool.tile([128, 512], F32, name="ost", tag="ost")
                nc.scalar.copy(ost[:], ps[:])
                nc.sync.dma_start(
                    out[qc * 128:(qc + 1) * 128, nh * 512:(nh + 1) * 512],
                    ost[:])
                proj_done.append((qc, nh))

            for pair in range(PAIRS):
                wts = wts0 if pair == 0 else dma_wqk(pair)
                qkt = {}
                for which in ("q", "k"):
                    qkt[(which, pair)] = qkt_pool.tile(
                        [128, N], F32R, name="qkt_t", tag="qkt_t")
                if pair == 1:
                    # prefetch proj weights once SBUF headroom exists
                    for pp_ in range(PAIRS):
                        t = wp_pool.tile([128, C], F32R, name="wp_t",
                                         tag="wp_t")
                        nc.sync.dma_start(
                            t[:], wp[pp_ * 128:(pp_ + 1) * 128, :])
                        wp_t.append(t)
                def qk_group(which, qb):
                    dst = qkt[(which, pair)]
                    ps = ps1_pool.tile([128, 512], F32, name="ps1t",
                                       tag="ps1t")
                    for cc in range(CC):
                        nc.tensor.matmul(
                            ps[:], wts[(which, cc)][:],
                            xt_q[(cc, qb)][:],
                            start=(cc == 0), stop=(cc == CC - 1))
                    nc.vector.tensor_copy(
                        dst[:, qb * 512:(qb + 1) * 512], ps[:])

                # K^T is contracted over ALL key blocks by every attention
                # block, so it must be complete before (or produced just
                # ahead of) the k-chunks that read it. For pair 0 the later
                # K-groups are injected into the first block's k-loop (with
                # V) so the exp stream starts after ~2 QK groups, not 5.
                if pair == 0:
                    qk_group("k", 0)

                    def pre0(kc):
                        if kc in (1, 5, 9):
                            qk_group("k", kc // 4 + 1)
                        emit_v(kc)
                else:
                    for qb in range(QB):
                        qk_group("k", qb)
                    pre0 = None
                for qb in range(QB):
                    qk_group("q", qb)
                    if pair == 0 and qb == 0:
                        pre = pre0
                    elif pair == PAIRS - 1 and qb >= 1:
                        # last pair: the q-blocks processed so far have
                        # complete xn across all pairs — stream their proj
                        # groups into this unit's k-loop (PE slack fills
                        # while ACT stays the critical engine)
                        done = set(proj_done)
                        lim, step = (8, 2) if qb == QB - 1 else (5, 3)
                        pend = [(c, n) for c in range(qb * 4)
                                for n in range(2) if (c, n) not in done][:lim]

                        def pre(kc, _p=pend, _s=step):
                            if _p and kc % _s == 1:
                                proj_group(*_p.pop(0))
                    else:
                        pre = None
                    mtiles = _load_mask(nc, m_pool, mT, qb)
                    _phase2_block(nc, qb, pair, qkt, v_t, xn, mtiles,
                                  s_pool, u_pool,
                                  e_pool, p_pool, rinv_pool, b_pool,
                                  pre_kc=pre)

            # remaining proj groups (qb3's q-chunks + any not streamed)
            done = set(proj_done)
            for qc in range(QC):
                for nh in range(2):
                    if (qc, nh) not in done:
                        proj_group(qc, nh)


def build():
    if _NC_CACHE:
        return _NC_CACHE[0]
    nc = bacc.Bacc("TRN2", target_bir_lowering=False, debug=False,
                   enable_asserts=False, num_devices=N_CORES)
    xT = nc.dram_tensor("xT", [QB, 128, CC * 512], BF16,
                        kind="ExternalInput").ap()
    wq = nc.dram_tensor("wq", [PAIRS, 128, CC * 128], BF16,
                        kind="ExternalInput").ap()
    wk = nc.dram_tensor("wk", [PAIRS, 128, CC * 128], BF16,
                        kind="ExternalInput").ap()
    wv = nc.dram_tensor("wv", [128, CC * 512], BF16,
                        kind="ExternalInput").ap()
    mT = nc.dram_tensor("mT", [QB, KC, 128, 512], BF16,
                        kind="ExternalInput").ap()
    wp = nc.dram_tensor("wp", [GW, C], F32R, kind="ExternalInput").ap()
    out = nc.dram_tensor("out", [N, C], F32, kind="ExternalOutput").ap()
    with tile.TileContext(nc) as tc:
        _emit(tc, xT, wq, wk, wv, mT, wp, out)
    nc.compile()
    _NC_CACHE.append(nc)
    return nc


def _tile4(a, rows, cols):
    """[R, Q] -> [Q//cols, R//rows, rows, cols] contiguous tiles so every
    device DMA is a single contiguous transfer."""
    R, Q = a.shape
    return np.ascontiguousarray(
        a.reshape(R // rows, rows, Q // cols, cols).transpose(0, 2, 1, 3)
         .transpose(1, 0, 2, 3))


def _pack_cc(a, cols):
    """[C, Q] -> [Q//cols, 128, (C//128)*cols]: per q-block, the 8
    contraction chunks side by side on 128 partitions (one contiguous DMA
    per q-block)."""
    R, Q = a.shape
    t = a.reshape(R // 128, 128, Q // cols, cols)      # [cc, p, qb, c]
    return np.ascontiguousarray(
        t.transpose(2, 1, 0, 3).reshape(Q // cols, 128, (R // 128) * cols))


def shard_inputs(joint_feature, mask, W_qkv, W_proj, b_proj):
    mT = _tile4(np.ascontiguousarray(mask[0, 0].T).astype(ml_dtypes.bfloat16),
                128, 512)
    in_maps = []
    for c in range(N_CORES):
        b, g = divmod(c, 2)
        lo, hi = g * GW, (g + 1) * GW
        in_maps.append({
            "xT": _pack_cc(np.ascontiguousarray(joint_feature[b].T)
                           .astype(ml_dtypes.bfloat16), 512),
            "wq": _pack_cc((W_qkv[:, lo:hi] * SCALE)
                           .astype(ml_dtypes.bfloat16), 128),
            "wk": _pack_cc(W_qkv[:, C + lo:C + hi]
                           .astype(ml_dtypes.bfloat16), 128),
            "wv": _pack_cc(W_qkv[:, 2 * C + lo:2 * C + hi]
                           .astype(ml_dtypes.bfloat16), 512)[0],
            "mT": mT,
            "wp": np.ascontiguousarray(W_proj[lo:hi, :]).astype(np.float32),
        })
    return in_maps


def kernel(joint_feature, mask, W_qkv, W_proj, b_proj):
    joint_feature = np.asarray(joint_feature, dtype=np.float32)
    mask = np.asarray(mask)
    W_qkv = np.asarray(W_qkv, dtype=np.float32)
    W_proj = np.asarray(W_proj, dtype=np.float32)
    b_proj = np.asarray(b_proj, dtype=np.float32)

    nc = build()
    in_maps = shard_inputs(joint_feature, mask, W_qkv, W_proj, b_proj)
    res = bass_utils.run_bass_kernel_spmd(nc, in_maps,
                                          core_ids=list(range(N_CORES)))
    out = np.empty((B, N, C), dtype=np.float32)
    for b in range(B):
        out[b] = res.results[2 * b]["out"] + res.results[2 * b + 1]["out"] \
            + b_proj
    return out


# revision 64
# speedup vs baseline: 1.0451x; 1.0038x over previous
"""Bass/Trainium2 kernel for nn_Attention_46566035423948.

Multi-head attention (B=4, N=2048, C=1024, H=16) on 8 NeuronCores.
Sharding: core c = (batch b = c//2, head-group g = c%2, 8 heads each).
Each core computes a partial projection output [N, C]; the host sums the
two head-group partials per batch and adds b_proj.

Per-core dataflow (everything in "key/channel-on-partition" layout so the
softmax denominator is a matmul reduction):
  phase 1: Q^T,K^T [512, 2048] fp32r (head pairs packed 64+64 in partition
           chunks), V [2048, 8*65] natural layout bf16 with a ones column
           per head, from bf16 xT and the W_qkv slices (SCALE pre-folded
           into Wq on host).
  phase 2: per (q-block 512, head-pair, k-chunk 128):
           S^T = K^T.T @ Q^T (row-tiled pair of fp32r matmuls)
           P^T = exp(S^T) * maskT (ScalarE exp PSUM->SBUF bf16, DVE mask)
           U  += V_aug.T @ P^T (M=65: row 64 accumulates the softmax
           denominators for free), then xn^T = U * broadcast(1/U[64]).
           Phase 1 is interleaved pair-by-pair under q-blocks 0-1 so the
           ScalarE exp pipeline (the throughput floor) starts early.
  phase 3: out = xn^T.T @ W_proj_slice (fp32r), staged through SBUF to
           DRAM. Proj groups for completed q-blocks are streamed into the
           last head-pair's k-loops; only the final q-block's groups run
           as a tail.
"""

import numpy as np
import ml_dtypes

import concourse.mybir as mybir
import concourse.tile as tile
from concourse import bacc
from concourse import bass_utils

N_CORES = 8
B, N, C, H = 4, 2048, 1024, 16
HS = C // H           # 64
SCALE = HS ** -0.5
HPC = 8               # heads per core
GW = HPC * HS         # 512: per-core head-group width
PAIRS = 4             # head pairs per core
CC = C // 128         # 8 contraction chunks over C
KC = N // 128         # 16 key chunks
QB = N // 512         # 4 query blocks of 512
QC = N // 128         # 16 query chunks of 128 (proj)

F32 = mybir.dt.float32
F32R = mybir.dt.float32r
BF16 = mybir.dt.bfloat16
EXP = mybir.ActivationFunctionType.Exp

_NC_CACHE = []


def _load_mask(nc, m_pool, mT, qb):
    """DMA the 16 [128, 512] bf16 mask tiles for one q-block."""
    tiles = []
    for kc in range(KC):
        mt = m_pool.tile([128, 512], BF16, name="m_t", tag="m_t")
        nc.sync.dma_start(mt[:], mT[qb, kc])
        tiles.append(mt)
    return tiles


def _phase2_block(nc, qb, pair, qkt, v_t, xn, mtiles,
                  s_pool, u_pool, e_pool, p_pool,
                  rinv_pool, b_pool, pre_kc=None):
    """Attention for one (q-block, head-pair): S^T, exp, mask, augmented PV
    (which also accumulates the softmax denominators in U row 64), then
    normalize into xn[pair][:, qb*512:(qb+1)*512]."""
    qs = slice(qb * 512, (qb + 1) * 512)
    KT = qkt[("k", pair)]
    QT = qkt[("q", pair)]
    U = u_pool.tile([128, 1024], F32, name="U", tag="U")
    h0 = 2 * pair * (HS + 1)
    h1 = (2 * pair + 1) * (HS + 1)
    SKEW = 5  # PV trails S by 5 k-chunks so the first S's of a unit issue
    # before PV(kc=0) blocks the PE stream on the previous unit's normalize
    p_tiles = {}

    def emit_pv(kc):
        P = p_tiles.pop(kc)
        nc.tensor.matmul(
            U[0:65, 0:512], v_t[kc][:, h0:h0 + 65],
            P[:, 0:512], start=(kc == 0), stop=(kc == KC - 1),
            tile_position=(0, 0), skip_group_check=True)
        nc.tensor.matmul(
            U[0:65, 512:1024], v_t[kc][:, h1:h1 + 65],
            P[:, 512:1024], start=(kc == 0), stop=(kc == KC - 1),
            tile_position=(0, 0), skip_group_check=True)

    for kc in range(KC + SKEW):
        if kc < KC:
            if pre_kc is not None:
                pre_kc(kc)
            ks = slice(kc * 128, (kc + 1) * 128)
            mt = mtiles[kc]
            S = s_pool.tile([128, 1024], F32, name="S", tag="S")
            nc.tensor.matmul(S[:, 0:512], KT[0:64, ks], QT[0:64, qs],
                             start=True, stop=True, tile_position=(0, 0))
            nc.tensor.matmul(S[:, 512:1024], KT[64:128, ks], QT[64:128, qs],
                             start=True, stop=True, tile_position=(64, 0))
            E = e_pool.tile([128, 1024], BF16, name="E", tag="E")
            nc.scalar.activation(E[:], S[:], EXP)
            P = p_pool.tile([128, 1024], BF16, name="P", tag="P")
            nc.vector.tensor_mul(P[:, 0:512], E[:, 0:512], mt[:])
            nc.vector.tensor_mul(P[:, 512:1024], E[:, 512:1024], mt[:])
            p_tiles[kc] = P
        if kc >= SKEW:
            emit_pv(kc - SKEW)
    r01 = rinv_pool.tile([1, 1024], F32, name="r01", tag="r01")
    nc.vector.reciprocal(r01[:], U[64:65, :])
    Bc = b_pool.tile([128, 1024], F32, name="Bc", tag="Bc")
    nc.gpsimd.partition_broadcast(Bc[:], r01[:])
    nc.vector.tensor_mul(xn[pair][0:64, qs], U[0:64, 0:512], Bc[0:64, 0:512])
    nc.vector.tensor_mul(xn[pair][64:128, qs], U[0:64, 512:1024],
                         Bc[0:64, 512:1024])


def _emit(tc, xT, wq, wk, wv, mT, wp, out):
    nc = tc.nc
    from contextlib import ExitStack

    with ExitStack() as stack:
        # persistent pools: V lives through phase 2, xn through phase 3
        v_pool = stack.enter_context(tc.tile_pool(name="vp", bufs=KC))
        xn_pool = stack.enter_context(tc.tile_pool(name="xn", bufs=1))
        wp_pool = stack.enter_context(tc.tile_pool(name="wpp", bufs=PAIRS))
        ostage_pool = stack.enter_context(tc.tile_pool(name="ostage", bufs=6))

        v_t = []
        xn = [xn_pool.tile([128, N], F32R, name=f"xn{i}", tag=f"xn{i}")
              for i in range(PAIRS)]
        wp_t = []

        # single fused region: QKV production for (pair, qb) is interleaved
        # directly ahead of the attention block that consumes it, so the
        # ScalarE exp stream starts almost immediately and PE fills ACT
        # stalls with projection work throughout.
        with tc.tile_pool(name="qkt", bufs=4) as qkt_pool, \
             tc.tile_pool(name="ep", bufs=4) as e_pool, \
             tc.tile_pool(name="pp", bufs=7) as p_pool, \
             tc.tile_pool(name="rinv", bufs=2) as rinv_pool, \
             tc.tile_pool(name="binv", bufs=2) as b_pool, \
             tc.tile_pool(name="xt", bufs=1) as xt_pool, \
             tc.tile_pool(name="wqk", bufs=4) as wqk_pool, \
             tc.tile_pool(name="wvp", bufs=1) as wv_pool, \
             tc.tile_pool(name="mp", bufs=8) as m_pool, \
             tc.tile_pool(name="ps2s", bufs=2, space="PSUM") as s_pool, \
             tc.tile_pool(name="ps1", bufs=2, space="PSUM") as ps1_pool, \
             tc.tile_pool(name="ps2u", bufs=1, space="PSUM") as u_pool:

            def dma_wqk(pair):
                wts = {}
                for which, wsrc in (("k", wk), ("q", wq)):
                    wt = wqk_pool.tile([128, CC * 128], BF16, name="wqk_t",
                                       tag="wqk_t")
                    nc.sync.dma_start(wt[:], wsrc[pair])
                    for cc in range(CC):
                        wts[(which, cc)] = wt[:, cc * 128:(cc + 1) * 128]
                return wts

            # DMA in PE-consumption order: pair-0 K weights, the 8 qb=0
            # xt chunks and wv first — the first K-group starts after
            # ~0.75 MB of DMA.
            def dma_w(which, wsrc, pair, wts):
                wt = wqk_pool.tile([128, CC * 128], BF16, name="wqk_t",
                                   tag="wqk_t")
                nc.sync.dma_start(wt[:], wsrc[pair])
                for cc in range(CC):
                    wts[(which, cc)] = wt[:, cc * 128:(cc + 1) * 128]

            wts0 = {}
            dma_w("k", wk, 0, wts0)
            xt_q = {}
            wv_t = []
            t = xt_pool.tile([128, CC * 512], BF16, name="xt_0")
            half = CC * 256
            nc.sync.dma_start(t[:, 0:half], xT[0, :, 0:half])
            nc.sync.dma_start(t[:, half:], xT[0, :, half:])
            for cc in range(CC):
                xt_q[(cc, 0)] = t[:, cc * 512:(cc + 1) * 512]
            dma_w("q", wq, 0, wts0)
            t = wv_pool.tile([128, CC * 512], BF16, name="wv_all")
            nc.sync.dma_start(t[:, 0:half], wv[:, 0:half])
            nc.sync.dma_start(t[:, half:], wv[:, half:])
            for cc in range(CC):
                wv_t.append(t[:, cc * 512:(cc + 1) * 512])
            for qb in range(1, QB):
                t = xt_pool.tile([128, CC * 512], BF16, name=f"xt_{qb}")
                nc.sync.dma_start(t[:], xT[qb])
                for cc in range(CC):
                    xt_q[(cc, qb)] = t[:, cc * 512:(cc + 1) * 512]

            # V tiles are emitted lazily inside the first attention block's
            # k-loop so ScalarE's exp pipeline starts early. Layout
            # [128, 8*65]: head h at cols h*65..h*65+64 plus a ones column
            # at h*65+64, so PV matmuls (M=65) also produce the softmax
            # row sums.
            def emit_v(kc):
                ps = ps1_pool.tile([128, 512], F32, name="ps1t", tag="ps1t")
                for cc in range(CC):
                    nc.tensor.matmul(
                        ps[:],
                        xt_q[(cc, kc // 4)][:, (kc % 4) * 128:
                                            (kc % 4) * 128 + 128],
                        wv_t[cc],
                        start=(cc == 0), stop=(cc == CC - 1))
                t = v_pool.tile([128, HPC * (HS + 1)], BF16, name="v_t",
                                tag="v_t")
                tv = t[:].rearrange("p (h d) -> p h d", h=HPC)
                nc.gpsimd.memset(tv[:, :, HS:HS + 1], 1.0)
                nc.vector.tensor_copy(
                    tv[:, :, 0:HS],
                    ps[:].rearrange("p (h d) -> p h d", h=HPC))
                v_t.append(t)

            proj_done = []

            def proj_group(qc, nh, pool=None):
                pool = pool if pool is not None else ps1_pool
                ps = pool.tile([128, 512], F32, name="ps1t", tag="ps1t")
                for pair_ in range(PAIRS):
                    nc.tensor.matmul(
                        ps[:], xn[pair_][:, qc * 128:(qc + 1) * 128],
                        wp_t[pair_][:, nh * 512:(nh + 1) * 512],
                        start=(pair_ == 0), stop=(pair_ == PAIRS - 1))
                ost = ostage_pool.tile([128, 512], F32, name="ost", tag="ost")
                nc.scalar.copy(ost[:], ps[:])
                nc.sync.dma_start(
                    out[qc * 128:(qc + 1) * 128, nh * 512:(nh + 1) * 512],
                    ost[:])
                proj_done.append((qc, nh))

            for pair in range(PAIRS):
                wts = wts0 if pair == 0 else dma_wqk(pair)
                qkt = {}
                for which in ("q", "k"):
                    qkt[(which, pair)] = qkt_pool.tile(
                        [128, N], F32R, name="qkt_t", tag="qkt_t")
                if pair == 1:
                    # prefetch proj weights once SBUF headroom exists
                    for pp_ in range(PAIRS):
                        t = wp_pool.tile([128, C], F32R, name="wp_t",
                                         tag="wp_t")
                        nc.sync.dma_start(
                            t[:], wp[pp_ * 128:(pp_ + 1) * 128, :])
                        wp_t.append(t)
                def qk_group(which, qb):
                    dst = qkt[(which, pair)]
                    ps = ps1_pool.tile([128, 512], F32, name="ps1t",
                                       tag="ps1t")
                    for cc in range(CC):
                        nc.tensor.matmul(
                            ps[:], wts[(which, cc)][:],
                            xt_q[(cc, qb)][:],
                            start=(cc == 0), stop=(cc == CC - 1))
                    nc.vector.tensor_copy(
                        dst[:, qb * 512:(qb + 1) * 512], ps[:])

                # K^T is contracted over ALL key blocks by every attention
                # block, so it must be complete before (or produced just
                # ahead of) the k-chunks that read it. For pair 0 the later
                # K-groups are injected into the first block's k-loop (with
                # V) so the exp stream starts after ~2 QK groups, not 5.
                if pair == 0:
                    qk_group("k", 0)

                    def pre0(kc):
                        if kc in (1, 5, 9):
                            qk_group("k", kc // 4 + 1)
                        emit_v(kc)
                else:
                    for qb in range(QB):
                        qk_group("k", qb)
                    pre0 = None
                for qb in range(QB):
                    qk_group("q", qb)
                    if pair == 0 and qb == 0:
                        pre = pre0
                    elif pair == PAIRS - 1 and qb >= 1:
                        # last pair: the q-blocks processed so far have
                        # complete xn across all pairs — stream their proj
                        # groups into this unit's k-loop (PE slack fills
                        # while ACT stays the critical engine)
                        done = set(proj_done)
                        lim, step = (8, 2) if qb == QB - 1 else (5, 3)
                        pend = [(c, n) for c in range(qb * 4)
                                for n in range(2) if (c, n) not in done][:lim]

                        def pre(kc, _p=pend, _s=step):
                            if _p and kc % _s == 1:
                                proj_group(*_p.pop(0))
                    else:
                        pre = None
                    mtiles = _load_mask(nc, m_pool, mT, qb)
                    _phase2_block(nc, qb, pair, qkt, v_t, xn, mtiles,
                                  s_pool, u_pool,
                                  e_pool, p_pool, rinv_pool, b_pool,
                                  pre_kc=pre)

            # remaining proj groups (qb3's q-chunks + any not streamed)
            done = set(proj_done)
            for qc in range(QC):
                for nh in range(2):
                    if (qc, nh) not in done:
                        proj_group(qc, nh)


def build():
    if _NC_CACHE:
        return _NC_CACHE[0]
    nc = bacc.Bacc("TRN2", target_bir_lowering=False, debug=False,
                   enable_asserts=False, num_devices=N_CORES)
    xT = nc.dram_tensor("xT", [QB, 128, CC * 512], BF16,
                        kind="ExternalInput").ap()
    wq = nc.dram_tensor("wq", [PAIRS, 128, CC * 128], BF16,
                        kind="ExternalInput").ap()
    wk = nc.dram_tensor("wk", [PAIRS, 128, CC * 128], BF16,
                        kind="ExternalInput").ap()
    wv = nc.dram_tensor("wv", [128, CC * 512], BF16,
                        kind="ExternalInput").ap()
    mT = nc.dram_tensor("mT", [QB, KC, 128, 512], BF16,
                        kind="ExternalInput").ap()
    wp = nc.dram_tensor("wp", [GW, C], F32R, kind="ExternalInput").ap()
    out = nc.dram_tensor("out", [N, C], F32, kind="ExternalOutput").ap()
    with tile.TileContext(nc) as tc:
        _emit(tc, xT, wq, wk, wv, mT, wp, out)
    nc.compile()
    _NC_CACHE.append(nc)
    return nc


def _tile4(a, rows, cols):
    """[R, Q] -> [Q//cols, R//rows, rows, cols] contiguous tiles so every
    device DMA is a single contiguous transfer."""
    R, Q = a.shape
    return np.ascontiguousarray(
        a.reshape(R // rows, rows, Q // cols, cols).transpose(0, 2, 1, 3)
         .transpose(1, 0, 2, 3))


def _pack_cc(a, cols):
    """[C, Q] -> [Q//cols, 128, (C//128)*cols]: per q-block, the 8
    contraction chunks side by side on 128 partitions (one contiguous DMA
    per q-block)."""
    R, Q = a.shape
    t = a.reshape(R // 128, 128, Q // cols, cols)      # [cc, p, qb, c]
    return np.ascontiguousarray(
        t.transpose(2, 1, 0, 3).reshape(Q // cols, 128, (R // 128) * cols))


def shard_inputs(joint_feature, mask, W_qkv, W_proj, b_proj):
    mT = _tile4(np.ascontiguousarray(mask[0, 0].T).astype(ml_dtypes.bfloat16),
                128, 512)
    in_maps = []
    for c in range(N_CORES):
        b, g = divmod(c, 2)
        lo, hi = g * GW, (g + 1) * GW
        in_maps.append({
            "xT": _pack_cc(np.ascontiguousarray(joint_feature[b].T)
                           .astype(ml_dtypes.bfloat16), 512),
            "wq": _pack_cc((W_qkv[:, lo:hi] * SCALE)
                           .astype(ml_dtypes.bfloat16), 128),
            "wk": _pack_cc(W_qkv[:, C + lo:C + hi]
                           .astype(ml_dtypes.bfloat16), 128),
            "wv": _pack_cc(W_qkv[:, 2 * C + lo:2 * C + hi]
                           .astype(ml_dtypes.bfloat16), 512)[0],
            "mT": mT,
            "wp": np.ascontiguousarray(W_proj[lo:hi, :]).astype(np.float32),
        })
    return in_maps


def kernel(joint_feature, mask, W_qkv, W_proj, b_proj):
    joint_feature = np.asarray(joint_feature, dtype=np.float32)
    mask = np.asarray(mask)
    W_qkv = np.asarray(W_qkv, dtype=np.float32)
    W_proj = np.asarray(W_proj, dtype=np.float32)
    b_proj = np.asarray(b_proj, dtype=np.float32)

    nc = build()
    in_maps = shard_inputs(joint_feature, mask, W_qkv, W_proj, b_proj)
    res = bass_utils.run_bass_kernel_spmd(nc, in_maps,
                                          core_ids=list(range(N_CORES)))
    out = np.empty((B, N, C), dtype=np.float32)
    for b in range(B):
        out[b] = res.results[2 * b]["out"] + res.results[2 * b + 1]["out"] \
            + b_proj
    return out
